# revision 1
# baseline (speedup 1.0000x reference)
"""Trainium2 Bass kernel for the bidirectional-attention module.

Math (per batch item):
    fa = relu(relu(a @ W1.T + b1) @ W2.T + b2)      # [La, F]
    fb = relu(relu(b @ W1.T + b1) @ W2.T + b2)      # [Lb, F]
    E = fa @ fb.T                                   # [La, Lb]
    beta  = softmax(E, axis=-1) @ b                 # [La, H]
    alpha = softmax(E.T, axis=-1) @ a               # [Lb, H]

Device strategy (data-parallel over batch, 8 items per core):
  - Everything is computed in "transposed MLP space": with a.T available
    (host-pretransposed), h.T = W1 @ a.T and f.T = W2 @ h.T chain with the
    contraction dim always on partitions -> zero on-chip transposes.
  - Both E [La,Lb] and E.T [Lb,La] are materialized by two PE passes over
    (fa.T, fb.T).  A single *constant* softmax shift (SHIFT) keeps exp()
    in range and cancels in both row- and column-softmax, so the exp'd
    S = exp(E - SHIFT) tiles serve directly as matmul lhsT operands:
      beta  = diag(1/rowsum(S))  . (S @ b)    lhsT = S.T tiles, rhs = b
      alpha = diag(1/rowsum(St)) . (St @ a)   lhsT = S  tiles, rhs = a
    The 1/sum scaling folds into the PSUM->SBUF epilogue as a per-partition
    scalar multiply.
  - exp() runs on the Scalar engine with accum_out giving rowsums for free;
    relu+bias epilogues run on the Vector engine (tensor_scalar add+max).
"""

import contextlib

import numpy as np

import concourse.bass as bass
import concourse.mybir as mybir
import concourse.tile as tile
from concourse import bacc
from concourse.bass_utils import run_bass_kernel_spmd

P = 128
B, L, H, F = 64, 512, 1024, 512
NCORES = 8
BPC = B // NCORES          # batch items per core
KH, KF, ML = H // P, F // P, L // P
NH = H // 512              # free-dim chunks for the attention output
SHIFT = 130.0              # global softmax shift; E in [27, 138] for these inputs

F32 = mybir.dt.float32

# dtype knobs (device compute dtypes).  float32r = fp32 storage, reduced-
# precision PE multiply at 1 cyc/row (vs 4 for full fp32) for N>=256.
MLP_DT = mybir.dt.float32r  # aT/bT, W1T/W2T, hT, fT  (MLP + E matmul operands)
ATT_DT = mybir.dt.float32r  # S/St, natural-layout a/b (attention matmul operands)
NP_MLP = np.float32
NP_ATT = np.float32


def _build_nc(repeat=1):
    nc = bacc.Bacc("TRN2", target_bir_lowering=False,
                   detect_race_conditions=False)

    aT = nc.dram_tensor("aT", [BPC, H, L], MLP_DT, kind="ExternalInput")
    bT = nc.dram_tensor("bT", [BPC, H, L], MLP_DT, kind="ExternalInput")
    an = nc.dram_tensor("an", [BPC, L, H], ATT_DT, kind="ExternalInput")
    bn = nc.dram_tensor("bn", [BPC, L, H], ATT_DT, kind="ExternalInput")
    w1T = nc.dram_tensor("w1T", [H, F], MLP_DT, kind="ExternalInput")
    w2T = nc.dram_tensor("w2T", [F, F], MLP_DT, kind="ExternalInput")
    bias1 = nc.dram_tensor("bias1", [F], F32, kind="ExternalInput")
    bias2 = nc.dram_tensor("bias2", [F], F32, kind="ExternalInput")
    beta = nc.dram_tensor("beta", [BPC, L, H], F32, kind="ExternalOutput")
    alpha = nc.dram_tensor("alpha", [BPC, L, H], F32, kind="ExternalOutput")

    ADD, MAX = mybir.AluOpType.add, mybir.AluOpType.max
    EXP = mybir.ActivationFunctionType.Exp

    def MM(out, lhsT, rhs, start, stop):
        nc.tensor.matmul(out, lhsT, rhs, start=start, stop=stop)

    with contextlib.ExitStack() as ctx:
        tc = ctx.enter_context(tile.TileContext(nc))
        consts = ctx.enter_context(tc.tile_pool(name="consts", bufs=1))
        inT_pool = ctx.enter_context(tc.tile_pool(name="inT", bufs=1))
        nat_pool = ctx.enter_context(tc.tile_pool(name="nat", bufs=1))
        mid_pool = ctx.enter_context(tc.tile_pool(name="mid", bufs=1))
        s_pool = ctx.enter_context(tc.tile_pool(name="spool", bufs=1))
        small = ctx.enter_context(tc.tile_pool(name="small", bufs=2))
        out_pool = ctx.enter_context(tc.tile_pool(name="outp", bufs=4))
        psum_pool = ctx.enter_context(tc.tile_pool(name="ps", bufs=4, space="PSUM"))
        psum_att = ctx.enter_context(tc.tile_pool(name="psatt", bufs=2, space="PSUM"))

        w1s = consts.tile([P, KH, F], MLP_DT)
        nc.sync.dma_start(out=w1s, in_=w1T.rearrange("(k p) f -> p k f", p=P))
        w2s = consts.tile([P, KF, F], MLP_DT)
        nc.sync.dma_start(out=w2s, in_=w2T.rearrange("(k p) f -> p k f", p=P))
        b1s = consts.tile([P, KF], F32)
        nc.sync.dma_start(out=b1s, in_=bias1.rearrange("(m p) -> p m", p=P))
        b2s = consts.tile([P, KF], F32)
        nc.sync.dma_start(out=b2s, in_=bias2.rearrange("(m p) -> p m", p=P))
        nshift = consts.tile([P, 1], F32)
        nc.vector.memset(nshift, -SHIFT)

        for i in [i for _ in range(repeat) for i in range(BPC)]:
            aTs = inT_pool.tile([P, KH, L], MLP_DT, tag="aTs")
            nc.sync.dma_start(out=aTs, in_=aT[i].rearrange("(k p) l -> p k l", p=P))
            bTs = inT_pool.tile([P, KH, L], MLP_DT, tag="bTs")
            nc.sync.dma_start(out=bTs, in_=bT[i].rearrange("(k p) l -> p k l", p=P))
            ans = nat_pool.tile([P, ML, H], ATT_DT, tag="ans")
            nc.sync.dma_start(out=ans, in_=an[i].rearrange("(m p) h -> p m h", p=P))
            bns = nat_pool.tile([P, ML, H], ATT_DT, tag="bns")
            nc.sync.dma_start(out=bns, in_=bn[i].rearrange("(m p) h -> p m h", p=P))

            # two-layer MLP, all in transposed space: fT = relu(W2 @ relu(W1 @ xT + b1) + b2)
            fTs = {}
            for name, xTs in (("a", aTs), ("b", bTs)):
                hts = mid_pool.tile([P, KF, L], MLP_DT, tag=f"h_{name}")
                for m in range(KF):
                    ps = psum_pool.tile([P, L], F32, tag="ps")
                    for k in range(KH):
                        MM(ps, w1s[:, k, m * P:(m + 1) * P],
                           xTs[:, k, :], start=(k == 0), stop=(k == KH - 1))
                    nc.vector.tensor_scalar(out=hts[:, m, :], in0=ps,
                                            scalar1=b1s[:, m:m + 1], scalar2=0.0,
                                            op0=ADD, op1=MAX)
                fts = mid_pool.tile([P, KF, L], MLP_DT, tag=f"f_{name}")
                for m in range(KF):
                    ps = psum_pool.tile([P, L], F32, tag="ps")
                    for k in range(KF):
                        MM(ps, w2s[:, k, m * P:(m + 1) * P],
                           hts[:, k, :], start=(k == 0), stop=(k == KF - 1))
                    nc.vector.tensor_scalar(out=fts[:, m, :], in0=ps,
                                            scalar1=b2s[:, m:m + 1], scalar2=0.0,
                                            op0=ADD, op1=MAX)
                fTs[name] = fts
            faT, fbT = fTs["a"], fTs["b"]

            # E and E.T, exp'd with the constant shift; rowsums via ACT accum
            Ss = s_pool.tile([P, ML, L], ATT_DT, tag="S")
            Sts = s_pool.tile([P, ML, L], ATT_DT, tag="St")
            rsum = small.tile([P, ML], F32, tag="rsum")
            csum = small.tile([P, ML], F32, tag="csum")
            for Sout, acc, lhs, rhs in ((Ss, rsum, faT, fbT), (Sts, csum, fbT, faT)):
                for m in range(ML):
                    ps = psum_pool.tile([P, L], F32, tag="ps")
                    for k in range(KF):
                        MM(ps, lhs[:, k, m * P:(m + 1) * P],
                           rhs[:, k, :], start=(k == 0), stop=(k == KF - 1))
                    nc.scalar.activation(out=Sout[:, m, :], in_=ps, func=EXP,
                                         bias=nshift, scale=1.0,
                                         accum_out=acc[:, m:m + 1])
            rinv = small.tile([P, ML], F32, tag="rinv")
            nc.vector.reciprocal(out=rinv, in_=rsum)
            cinv = small.tile([P, ML], F32, tag="cinv")
            nc.vector.reciprocal(out=cinv, in_=csum)

            # beta = diag(rinv) . (S @ b);  alpha = diag(cinv) . (St @ a)
            for out_dram, lhsS, rhs_nat, inv in ((beta, Sts, bns, rinv),
                                                 (alpha, Ss, ans, cinv)):
                for m in range(ML):
                    ps2 = psum_att.tile([P, H], F32, tag="psatt")
                    for nh in range(NH):
                        for k in range(ML):
                            MM(ps2[:, nh * 512:(nh + 1) * 512],
                               lhsS[:, k, m * P:(m + 1) * P],
                               rhs_nat[:, k, nh * 512:(nh + 1) * 512],
                               start=(k == 0), stop=(k == ML - 1))
                    ot = out_pool.tile([P, H], F32, tag="ot")
                    nc.vector.tensor_scalar(out=ot, in0=ps2, scalar1=inv[:, m:m + 1],
                                            scalar2=None, op0=mybir.AluOpType.mult)
                    nc.sync.dma_start(out=out_dram[i, m * P:(m + 1) * P, :], in_=ot)
    nc.compile()
    return nc


_NC_CACHE = {}


def _get_nc(repeat=1):
    if repeat not in _NC_CACHE:
        _NC_CACHE[repeat] = _build_nc(repeat)
    return _NC_CACHE[repeat]


def kernel(a, b, W1, b1, W2, b2):
    a = np.ascontiguousarray(np.asarray(a, dtype=np.float32))
    b = np.ascontiguousarray(np.asarray(b, dtype=np.float32))
    w1T_h = np.ascontiguousarray(np.asarray(W1, np.float32).T.astype(NP_MLP))
    w2T_h = np.ascontiguousarray(np.asarray(W2, np.float32).T.astype(NP_MLP))
    b1_h = np.ascontiguousarray(np.asarray(b1, np.float32))
    b2_h = np.ascontiguousarray(np.asarray(b2, np.float32))

    in_maps = []
    for c in range(NCORES):
        sl = slice(c * BPC, (c + 1) * BPC)
        ac, bc = a[sl], b[sl]
        in_maps.append({
            "aT": np.ascontiguousarray(ac.transpose(0, 2, 1)).astype(NP_MLP),
            "bT": np.ascontiguousarray(bc.transpose(0, 2, 1)).astype(NP_MLP),
            "an": ac.astype(NP_ATT),
            "bn": bc.astype(NP_ATT),
            "w1T": w1T_h,
            "w2T": w2T_h,
            "bias1": b1_h,
            "bias2": b2_h,
        })

    res = run_bass_kernel_spmd(_get_nc(), in_maps, core_ids=list(range(NCORES)))
    beta = np.concatenate([res.results[c]["beta"] for c in range(NCORES)], axis=0)
    alpha = np.concatenate([res.results[c]["alpha"] for c in range(NCORES)], axis=0)
    return beta.astype(np.float32), alpha.astype(np.float32)



# revision 20
# speedup vs baseline: 1.0958x; 1.0958x over previous
"""Trainium2 Bass kernel for the bidirectional-attention module.

Math (per batch item):
    fa = relu(relu(a @ W1.T + b1) @ W2.T + b2)      # [La, F]
    fb = relu(relu(b @ W1.T + b1) @ W2.T + b2)      # [Lb, F]
    E = fa @ fb.T                                   # [La, Lb]
    beta  = softmax(E, axis=-1) @ b                 # [La, H]
    alpha = softmax(E.T, axis=-1) @ a               # [Lb, H]

Device strategy (data-parallel over batch, 8 items per core):
  - MLP in "transposed space" (h.T = W1 @ a.T etc., contraction on
    partitions, fp32r at 1 cyc/row) -> faT/fbT.
  - E computed once per item via PE matmul (fp32r); E.T via PE transpose
    of E (1.5 cyc/row) instead of a second matmul pass.
  - Per-row softmax on each side: Sb = exp(E - rowmax) [a on partitions]
    and Stb = exp(E.T - colmax) [c on partitions], both bf16.  With
    per-row shifts these are NOT transposes of each other, and each
    attention matmul needs the *other* orientation for its stationary
    operand, so each is PE-transposed (bf16, 1 cyc/row) and cast to
    fp8e4 into the DoubleRow lhsT tiles (both slots duplicated; the
    slices [:, k, m, :, :] are contiguous 256-byte runs as the HW
    weight loader requires).
  - Attention application on the PE in fp8e4 DoubleRow perf mode
    (0.5 cyc/row): slots hold (S, S) x (b_hi, b_lo) where b = b_hi+b_lo
    is a host-side fp8 residual split, so the slot-sum reconstructs
    S @ b with ~6e-4 effective rhs precision; only S's own fp8
    quantization (~0.6% on the output) remains.
  - Softmax sums are taken over the fp8-quantized values (fp8 cast +
    DVE reduce) so numerator/denominator quantization cancels on
    peaked rows.
  - Software pipelining: the previous item's two attention halves are
    issued inside the current item's softmax chains so the PE never
    head-of-line blocks on ACT/DVE.
  - PE work/item: MLP 49152 + E 8192 + E.T transposes 3072 + S/St
    transposes 4096 + attention 16384 = 80896 cycles (vs 98304 for the
    all-fp32r baseline, -18%).
  - Outputs are written bf16 and upcast to fp32 on the host.
"""

import contextlib

import numpy as np

import concourse.bass as bass
import concourse.mybir as mybir
import concourse.tile as tile
from concourse import bacc
from concourse import masks
from concourse.bass_utils import run_bass_kernel_spmd

P = 128
B, L, H, F = 64, 512, 1024, 512
NCORES = 8
BPC = B // NCORES          # batch items per core
KH, KF, ML = H // P, F // P, L // P
NH = H // 512              # free-dim chunks for the attention output

F32 = mybir.dt.float32
F32R = mybir.dt.float32r   # fp32 storage, 1 cyc/row PE for free dim >= 256
BF16 = mybir.dt.bfloat16
FP8 = mybir.dt.float8e4
NP_MLP = np.float32
NP_FP8 = mybir.dt.np(FP8)

DR = mybir.MatmulPerfMode.DoubleRow
EXP = mybir.ActivationFunctionType.Exp
RELU = mybir.ActivationFunctionType.Relu
MAX = mybir.AluOpType.max
ADD = mybir.AluOpType.add
MULT = mybir.AluOpType.mult


def _build_nc(repeat=1):
    nc = bacc.Bacc("TRN2", target_bir_lowering=False,
                   detect_race_conditions=False)

    aT = nc.dram_tensor("aT", [BPC, H, L], F32R, kind="ExternalInput")
    bT = nc.dram_tensor("bT", [BPC, H, L], F32R, kind="ExternalInput")
    a8 = nc.dram_tensor("a8", [BPC, L, NH, 2, 512], FP8, kind="ExternalInput")
    b8 = nc.dram_tensor("b8", [BPC, L, NH, 2, 512], FP8, kind="ExternalInput")
    w1T = nc.dram_tensor("w1T", [H, F], F32R, kind="ExternalInput")
    w2T = nc.dram_tensor("w2T", [F, F], F32R, kind="ExternalInput")
    bias1 = nc.dram_tensor("bias1", [F], F32, kind="ExternalInput")
    bias2 = nc.dram_tensor("bias2", [F], F32, kind="ExternalInput")
    beta = nc.dram_tensor("beta", [BPC, L, H], BF16, kind="ExternalOutput")
    alpha = nc.dram_tensor("alpha", [BPC, L, H], BF16, kind="ExternalOutput")

    def MM(out, lhsT, rhs, start, stop):
        nc.tensor.matmul(out, lhsT, rhs, start=start, stop=stop)

    with contextlib.ExitStack() as ctx:
        tc = ctx.enter_context(tile.TileContext(nc))
        consts = ctx.enter_context(tc.tile_pool(name="consts", bufs=1))
        inT_pool = ctx.enter_context(tc.tile_pool(name="inT", bufs=2))
        nat_pool = ctx.enter_context(tc.tile_pool(name="nat", bufs=2))
        mid_pool = ctx.enter_context(tc.tile_pool(name="mid", bufs=1))
        e_pool = ctx.enter_context(tc.tile_pool(name="epool", bufs=1))
        s_pool = ctx.enter_context(tc.tile_pool(name="spool", bufs=1))
        ts_pool = ctx.enter_context(tc.tile_pool(name="tspool", bufs=2))
        small = ctx.enter_context(tc.tile_pool(name="small", bufs=4))
        out_pool = ctx.enter_context(tc.tile_pool(name="outp", bufs=4))
        psum_pool = ctx.enter_context(tc.tile_pool(name="ps", bufs=3, space="PSUM"))
        psum_att = ctx.enter_context(tc.tile_pool(name="psatt", bufs=2, space="PSUM"))

        w1s = consts.tile([P, KH, F], F32R)
        nc.sync.dma_start(out=w1s, in_=w1T.rearrange("(k p) f -> p k f", p=P))
        w2s = consts.tile([P, KF, F], F32R)
        nc.sync.dma_start(out=w2s, in_=w2T.rearrange("(k p) f -> p k f", p=P))
        b1s = consts.tile([P, KF], F32)
        nc.sync.dma_start(out=b1s, in_=bias1.rearrange("(m p) -> p m", p=P))
        b2s = consts.tile([P, KF], F32)
        nc.sync.dma_start(out=b2s, in_=bias2.rearrange("(m p) -> p m", p=P))
        ident_f32 = consts.tile([P, P], F32)
        masks.make_identity(nc, ident_f32)
        ident = consts.tile([P, P], F32R)
        nc.vector.tensor_copy(ident, ident_f32)
        identb = consts.tile([P, P], BF16)
        nc.vector.tensor_copy(identb, ident_f32)

        def emit_attention_half(st, which):
            """One output's attention matmuls + epilogue for a prior item."""
            if which == 0:
                out_dram, lhs8, rhs8, sums, tag = (beta, st["TSb"], st["b8s"],
                                                   st["rsum"], "rinv")
            else:
                out_dram, lhs8, rhs8, sums, tag = (alpha, st["TSa"], st["a8s"],
                                                   st["csum"], "cinv")
            inv = small.tile([P, ML], F32, tag=tag)
            nc.vector.reciprocal(out=inv, in_=sums)
            for m in range(ML):
                ps2 = psum_att.tile([P, H], F32, tag="psatt")
                for nh in range(NH):
                    for k in range(ML):
                        nc.tensor.matmul(
                            ps2[:, nh * 512:(nh + 1) * 512],
                            lhs8[:, k, m, :, :],
                            rhs8[:, k, nh, :, :],
                            start=(k == 0), stop=(k == ML - 1),
                            perf_mode=DR)
                ot = out_pool.tile([P, H], BF16, tag="ot")
                nc.vector.tensor_scalar(out=ot, in0=ps2,
                                        scalar1=inv[:, m:m + 1],
                                        scalar2=None, op0=MULT)
                nc.sync.dma_start(out=out_dram[st["i"], m * P:(m + 1) * P, :],
                                  in_=ot)

        prev = None
        for i in [i for _ in range(repeat) for i in range(BPC)]:
            aTs = inT_pool.tile([P, KH, L], F32R, tag="aTs")
            nc.sync.dma_start(out=aTs, in_=aT[i].rearrange("(k p) l -> p k l", p=P))
            bTs = inT_pool.tile([P, KH, L], F32R, tag="bTs")
            nc.sync.dma_start(out=bTs, in_=bT[i].rearrange("(k p) l -> p k l", p=P))
            a8s = nat_pool.tile([P, ML, NH, 2, 512], FP8, tag="a8s")
            b8s = nat_pool.tile([P, ML, NH, 2, 512], FP8, tag="b8s")
            for k in range(ML):
                nc.sync.dma_start(out=a8s[:, k], in_=a8[i, k * P:(k + 1) * P])
                nc.sync.dma_start(out=b8s[:, k], in_=b8[i, k * P:(k + 1) * P])

            # two-layer MLP, transposed space: fT = relu(W2 @ relu(W1 @ xT + b1) + b2)
            fTs = {}
            for name, xTs in (("a", aTs), ("b", bTs)):
                hts = mid_pool.tile([P, KF, L], F32R, tag=f"h_{name}")
                for m in range(KF):
                    ps = psum_pool.tile([P, L], F32, tag="ps")
                    for k in range(KH):
                        MM(ps, w1s[:, k, m * P:(m + 1) * P],
                           xTs[:, k, :], start=(k == 0), stop=(k == KH - 1))
                    nc.scalar.activation(out=hts[:, m, :], in_=ps, func=RELU,
                                         bias=b1s[:, m:m + 1], scale=1.0)
                fts = mid_pool.tile([P, KF, L], F32R, tag=f"f_{name}")
                for m in range(KF):
                    ps = psum_pool.tile([P, L], F32, tag="ps")
                    for k in range(KF):
                        MM(ps, w2s[:, k, m * P:(m + 1) * P],
                           hts[:, k, :], start=(k == 0), stop=(k == KF - 1))
                    nc.scalar.activation(out=fts[:, m, :], in_=ps, func=RELU,
                                         bias=b2s[:, m:m + 1], scale=1.0)
                fTs[name] = fts
            faT, fbT = fTs["a"], fTs["b"]

            # E tiles (PE, fp32r); row stats; row-shifted exp -> Sb (bf16)
            E_sb = e_pool.tile([P, ML, L], F32R, tag="E")
            Sb = s_pool.tile([P, ML, L], BF16, tag="Sb")
            Stb = s_pool.tile([P, ML, L], BF16, tag="Stb")
            Sf8 = s_pool.tile([P, ML, L], FP8, tag="Sf8")
            Stf8 = s_pool.tile([P, ML, L], FP8, tag="Stf8")
            TSb = ts_pool.tile([P, ML, ML, 2, P], FP8, tag="TSb")
            TSa = ts_pool.tile([P, ML, ML, 2, P], FP8, tag="TSa")
            nrmax = small.tile([P, ML], F32, tag="nrmax")
            ncmax = small.tile([P, ML], F32, tag="ncmax")
            rsum = small.tile([P, ML], F32, tag="rsum")
            csum = small.tile([P, ML], F32, tag="csum")

            for m in range(ML):
                ps = psum_pool.tile([P, L], F32, tag="ps")
                for k in range(KF):
                    MM(ps, faT[:, k, m * P:(m + 1) * P],
                       fbT[:, k, :], start=(k == 0), stop=(k == KF - 1))
                nc.vector.tensor_reduce(out=nrmax[:, m:m + 1], in_=ps,
                                        axis=mybir.AxisListType.X, op=MAX,
                                        negate=True)
                nc.scalar.copy(out=E_sb[:, m, :], in_=ps)
                nc.scalar.activation(out=Sb[:, m, :], in_=ps, func=EXP,
                                     bias=nrmax[:, m:m + 1], scale=1.0)

            # overlap: previous item's beta attention fills the PE while this
            # item's ACT chain runs
            if prev is not None:
                emit_attention_half(prev, 0)

            # E.T via PE transpose (fp32r); col-shifted exp -> Stb (bf16)
            for m in range(ML):
                pst = psum_pool.tile([P, L], F32R, tag="ps")
                for j in range(ML):
                    nc.tensor.transpose(pst[:, j * P:(j + 1) * P],
                                        E_sb[:, j, m * P:(m + 1) * P], ident)
                nc.vector.tensor_reduce(out=ncmax[:, m:m + 1], in_=pst,
                                        axis=mybir.AxisListType.X, op=MAX,
                                        negate=True)
                nc.scalar.activation(out=Stb[:, m, :], in_=pst, func=EXP,
                                     bias=ncmax[:, m:m + 1], scale=1.0)

            # TSb = transpose(Sb) as fp8 DR weights [c-part]; beta lhsT
            for mt in range(ML):
                pst2 = psum_pool.tile([P, L], BF16, tag="ps2", bufs=1)
                for j in range(ML):
                    nc.tensor.transpose(pst2[:, j * P:(j + 1) * P],
                                        Sb[:, j, mt * P:(mt + 1) * P], identb)
                for s in range(2):
                    nc.scalar.copy(out=TSb[:, mt, :, s, :], in_=pst2)

            # previous item's alpha attention covers the Stb/cast chains
            if prev is not None:
                emit_attention_half(prev, 1)

            # TSa = transpose(Stb) as fp8 DR weights [a-part]; alpha lhsT
            for mt in range(ML):
                pst2 = psum_pool.tile([P, L], BF16, tag="ps2", bufs=1)
                for j in range(ML):
                    nc.tensor.transpose(pst2[:, j * P:(j + 1) * P],
                                        Stb[:, j, mt * P:(mt + 1) * P], identb)
                for s in range(2):
                    nc.scalar.copy(out=TSa[:, mt, :, s, :], in_=pst2)

            # fp8 casts (quantization-consistent softmax sums)
            for m in range(ML):
                nc.scalar.copy(out=Sf8[:, m, :], in_=Sb[:, m, :])
                nc.vector.tensor_reduce(out=rsum[:, m:m + 1], in_=Sf8[:, m, :],
                                        axis=mybir.AxisListType.X, op=ADD)
                nc.scalar.copy(out=Stf8[:, m, :], in_=Stb[:, m, :])
                nc.vector.tensor_reduce(out=csum[:, m:m + 1], in_=Stf8[:, m, :],
                                        axis=mybir.AxisListType.X, op=ADD)

            prev = {"i": i, "TSb": TSb, "TSa": TSa, "a8s": a8s, "b8s": b8s,
                    "rsum": rsum, "csum": csum}
        emit_attention_half(prev, 0)
        emit_attention_half(prev, 1)
    nc.compile()
    return nc


_NC_CACHE = {}


def _get_nc(repeat=1):
    if repeat not in _NC_CACHE:
        _NC_CACHE[repeat] = _build_nc(repeat)
    return _NC_CACHE[repeat]


def _split_fp8(x):
    hi = x.astype(NP_FP8)
    lo = (x - hi.astype(np.float32)).astype(NP_FP8)
    # [BPC, L, NH, 2, 512]: per (row, nh) the hi/lo halves sit adjacent so the
    # DoubleRow ifmap slice [:, k, nh, :, :] is one contiguous 1024-run
    hi = hi.reshape(hi.shape[0], hi.shape[1], NH, 512)
    lo = lo.reshape(lo.shape[0], lo.shape[1], NH, 512)
    return np.stack([hi, lo], axis=3)


def build_in_maps(a, b, W1, b1, W2, b2):
    a = np.ascontiguousarray(np.asarray(a, dtype=np.float32))
    b = np.ascontiguousarray(np.asarray(b, dtype=np.float32))
    w1T_h = np.ascontiguousarray(np.asarray(W1, np.float32).T)
    w2T_h = np.ascontiguousarray(np.asarray(W2, np.float32).T)
    b1_h = np.ascontiguousarray(np.asarray(b1, np.float32))
    b2_h = np.ascontiguousarray(np.asarray(b2, np.float32))

    in_maps = []
    for c in range(NCORES):
        sl = slice(c * BPC, (c + 1) * BPC)
        ac, bc = a[sl], b[sl]
        in_maps.append({
            "aT": np.ascontiguousarray(ac.transpose(0, 2, 1)),
            "bT": np.ascontiguousarray(bc.transpose(0, 2, 1)),
            "a8": np.ascontiguousarray(_split_fp8(ac)),
            "b8": np.ascontiguousarray(_split_fp8(bc)),
            "w1T": w1T_h,
            "w2T": w2T_h,
            "bias1": b1_h,
            "bias2": b2_h,
        })
    return in_maps


def kernel(a, b, W1, b1, W2, b2):
    in_maps = build_in_maps(a, b, W1, b1, W2, b2)
    res = run_bass_kernel_spmd(_get_nc(), in_maps, core_ids=list(range(NCORES)))
    beta = np.concatenate([res.results[c]["beta"] for c in range(NCORES)], axis=0)
    alpha = np.concatenate([res.results[c]["alpha"] for c in range(NCORES)], axis=0)
    return beta.astype(np.float32), alpha.astype(np.float32)


# revision 26
# speedup vs baseline: 1.3206x; 1.2051x over previous
"""Trainium2 Bass kernel for the bidirectional-attention module.

Math (per batch item):
    fa = relu(relu(a @ W1.T + b1) @ W2.T + b2)      # [La, F]
    fb = relu(relu(b @ W1.T + b1) @ W2.T + b2)      # [Lb, F]
    E = fa @ fb.T                                   # [La, Lb]
    beta  = softmax(E, axis=-1) @ b                 # [La, H]
    alpha = softmax(E.T, axis=-1) @ a               # [Lb, H]

Device strategy (data-parallel over batch, 8 items per core):
  - MLP in "transposed space" (h.T = W1 @ a.T etc., contraction on
    partitions, fp32r at 1 cyc/row) -> faT/fbT.
  - E computed once per item via PE matmul (fp32r); E.T via PE transpose
    of E (1.5 cyc/row) instead of a second matmul pass.
  - Per-row softmax on each side: Sb = exp(E - rowmax) [a on partitions]
    and Stb = exp(E.T - colmax) [c on partitions], both bf16.  With
    per-row shifts these are NOT transposes of each other, and each
    attention matmul needs the *other* orientation for its stationary
    operand, so each is PE-transposed (bf16, 1 cyc/row) and cast to
    fp8e4 into the DoubleRow lhsT tiles (both slots duplicated; the
    slices [:, k, m, :, :] are contiguous 256-byte runs as the HW
    weight loader requires).
  - Attention application on the PE in fp8e4 DoubleRow perf mode
    (0.5 cyc/row): slots hold (S, S) x (b_hi, b_lo) where b = b_hi+b_lo
    is a host-side fp8 residual split, so the slot-sum reconstructs
    S @ b with ~6e-4 effective rhs precision; only S's own fp8
    quantization (~0.6% on the output) remains.
  - Softmax sums are taken over the fp8-quantized values (fp8 cast +
    DVE reduce) so numerator/denominator quantization cancels on
    peaked rows.
  - Software pipelining: the previous item's two attention halves are
    issued inside the current item's softmax chains so the PE never
    head-of-line blocks on ACT/DVE.
  - PE work/item: MLP 49152 + E 8192 + E.T transposes 3072 + S/St
    transposes 4096 + attention 16384 = 80896 cycles (vs 98304 for the
    all-fp32r baseline, -18%).
  - Outputs are written bf16 and upcast to fp32 on the host.
"""

import contextlib

import numpy as np

import concourse.bass as bass
import concourse.mybir as mybir
import concourse.tile as tile
from concourse import bacc
from concourse import masks
from concourse.bass_utils import run_bass_kernel_spmd

P = 128
B, L, H, F = 64, 512, 1024, 512
NCORES = 8
BPC = B // NCORES          # batch items per core
KH, KF, ML = H // P, F // P, L // P
NH = H // 512              # free-dim chunks for the attention output

F32 = mybir.dt.float32
F32R = mybir.dt.float32r   # fp32 storage, 1 cyc/row PE for free dim >= 256
BF16 = mybir.dt.bfloat16
FP8 = mybir.dt.float8e4
NP_MLP = np.float32
NP_FP8 = mybir.dt.np(FP8)

DR = mybir.MatmulPerfMode.DoubleRow
EXP = mybir.ActivationFunctionType.Exp
RELU = mybir.ActivationFunctionType.Relu
MAX = mybir.AluOpType.max
ADD = mybir.AluOpType.add
MULT = mybir.AluOpType.mult


def _build_nc(repeat=1):
    nc = bacc.Bacc("TRN2", target_bir_lowering=False,
                   detect_race_conditions=False)

    aT = nc.dram_tensor("aT", [BPC, H, L], F32R, kind="ExternalInput")
    bT = nc.dram_tensor("bT", [BPC, H, L], F32R, kind="ExternalInput")
    a8 = nc.dram_tensor("a8", [BPC, L, NH, 2, 512], FP8, kind="ExternalInput")
    b8 = nc.dram_tensor("b8", [BPC, L, NH, 2, 512], FP8, kind="ExternalInput")
    w1T = nc.dram_tensor("w1T", [H, F], F32R, kind="ExternalInput")
    w2T = nc.dram_tensor("w2T", [F, F], F32R, kind="ExternalInput")
    bias1 = nc.dram_tensor("bias1", [F], F32, kind="ExternalInput")
    bias2 = nc.dram_tensor("bias2", [F], F32, kind="ExternalInput")
    beta = nc.dram_tensor("beta", [BPC, L, H], BF16, kind="ExternalOutput")
    alpha = nc.dram_tensor("alpha", [BPC, L, H], BF16, kind="ExternalOutput")

    def MM(out, lhsT, rhs, start, stop):
        nc.tensor.matmul(out, lhsT, rhs, start=start, stop=stop)

    with contextlib.ExitStack() as ctx:
        tc = ctx.enter_context(tile.TileContext(nc))
        consts = ctx.enter_context(tc.tile_pool(name="consts", bufs=1))
        inT_pool = ctx.enter_context(tc.tile_pool(name="inT", bufs=2))
        nat_pool = ctx.enter_context(tc.tile_pool(name="nat", bufs=2))
        mid_pool = ctx.enter_context(tc.tile_pool(name="mid", bufs=1))
        e_pool = ctx.enter_context(tc.tile_pool(name="epool", bufs=1))
        s_pool = ctx.enter_context(tc.tile_pool(name="spool", bufs=1))
        ts_pool = ctx.enter_context(tc.tile_pool(name="tspool", bufs=2))
        small = ctx.enter_context(tc.tile_pool(name="small", bufs=4))
        out_pool = ctx.enter_context(tc.tile_pool(name="outp", bufs=4))
        psum_pool = ctx.enter_context(tc.tile_pool(name="ps", bufs=3, space="PSUM"))
        psum_att = ctx.enter_context(tc.tile_pool(name="psatt", bufs=2, space="PSUM"))

        w1s = consts.tile([P, KH, F], F32R)
        nc.sync.dma_start(out=w1s, in_=w1T.rearrange("(k p) f -> p k f", p=P))
        w2s = consts.tile([P, KF, F], F32R)
        nc.sync.dma_start(out=w2s, in_=w2T.rearrange("(k p) f -> p k f", p=P))
        b1s = consts.tile([P, KF], F32)
        nc.sync.dma_start(out=b1s, in_=bias1.rearrange("(m p) -> p m", p=P))
        b2s = consts.tile([P, KF], F32)
        nc.sync.dma_start(out=b2s, in_=bias2.rearrange("(m p) -> p m", p=P))
        ident_f32 = consts.tile([P, P], F32)
        masks.make_identity(nc, ident_f32)
        ident = consts.tile([P, P], F32R)
        nc.vector.tensor_copy(ident, ident_f32)
        ident8 = consts.tile([P, P], FP8)
        nc.vector.tensor_copy(ident8, ident_f32)

        def emit_attention_half(st, which):
            """One output's attention matmuls + epilogue for a prior item."""
            if which == 0:
                out_dram, lhs8, rhs8, sums, tag = (beta, st["TSb"], st["b8s"],
                                                   st["rsum"], "rinv")
            else:
                out_dram, lhs8, rhs8, sums, tag = (alpha, st["TSa"], st["a8s"],
                                                   st["csum"], "cinv")
            inv = small.tile([P, ML], F32, tag=tag)
            nc.vector.reciprocal(out=inv, in_=sums)
            for m in range(ML):
                ps2 = psum_att.tile([P, H], F32, tag="psatt")
                for nh in range(NH):
                    for k in range(ML):
                        nc.tensor.matmul(
                            ps2[:, nh * 512:(nh + 1) * 512],
                            lhs8[:, k, m, :, :],
                            rhs8[:, k, nh, :, :],
                            start=(k == 0), stop=(k == ML - 1),
                            perf_mode=DR)
                ot = out_pool.tile([P, H], BF16, tag="ot")
                nc.vector.tensor_scalar(out=ot, in0=ps2,
                                        scalar1=inv[:, m:m + 1],
                                        scalar2=None, op0=MULT)
                nc.sync.dma_start(out=out_dram[st["i"], m * P:(m + 1) * P, :],
                                  in_=ot)

        prev = None
        for i in [i for _ in range(repeat) for i in range(BPC)]:
            aTs = inT_pool.tile([P, KH, L], F32R, tag="aTs")
            nc.sync.dma_start(out=aTs, in_=aT[i].rearrange("(k p) l -> p k l", p=P))
            bTs = inT_pool.tile([P, KH, L], F32R, tag="bTs")
            nc.sync.dma_start(out=bTs, in_=bT[i].rearrange("(k p) l -> p k l", p=P))
            a8s = nat_pool.tile([P, ML, NH, 2, 512], FP8, tag="a8s")
            b8s = nat_pool.tile([P, ML, NH, 2, 512], FP8, tag="b8s")
            for k in range(ML):
                nc.sync.dma_start(out=a8s[:, k], in_=a8[i, k * P:(k + 1) * P])
                nc.sync.dma_start(out=b8s[:, k], in_=b8[i, k * P:(k + 1) * P])

            # two-layer MLP, transposed space: fT = relu(W2 @ relu(W1 @ xT + b1) + b2)
            fTs = {}
            for name, xTs in (("a", aTs), ("b", bTs)):
                hts = mid_pool.tile([P, KF, L], F32R, tag=f"h_{name}")
                for m in range(KF):
                    ps = psum_pool.tile([P, L], F32, tag="ps")
                    for k in range(KH):
                        MM(ps, w1s[:, k, m * P:(m + 1) * P],
                           xTs[:, k, :], start=(k == 0), stop=(k == KH - 1))
                    nc.scalar.activation(out=hts[:, m, :], in_=ps, func=RELU,
                                         bias=b1s[:, m:m + 1], scale=1.0)
                fts = mid_pool.tile([P, KF, L], F32R, tag=f"f_{name}")
                for m in range(KF):
                    ps = psum_pool.tile([P, L], F32, tag="ps")
                    for k in range(KF):
                        MM(ps, w2s[:, k, m * P:(m + 1) * P],
                           hts[:, k, :], start=(k == 0), stop=(k == KF - 1))
                    nc.scalar.activation(out=fts[:, m, :], in_=ps, func=RELU,
                                         bias=b2s[:, m:m + 1], scale=1.0)
                fTs[name] = fts
            faT, fbT = fTs["a"], fTs["b"]

            # E tiles (PE, fp32r); row stats; row-shifted exp -> Sf8 (fp8)
            Sf8 = s_pool.tile([P, ML, L], FP8, tag="Sf8")
            Stf8 = s_pool.tile([P, ML, L], FP8, tag="Stf8")
            TSb = ts_pool.tile([P, ML, ML, 2, P], FP8, tag="TSb")
            TSa = ts_pool.tile([P, ML, ML, 2, P], FP8, tag="TSa")
            nrmax = small.tile([P, ML], F32, tag="nrmax")
            ncmax = small.tile([P, ML], F32, tag="ncmax")
            rsum = small.tile([P, ML], F32, tag="rsum")
            csum = small.tile([P, ML], F32, tag="csum")

            for m in range(ML):
                ps = psum_pool.tile([P, L], F32, tag="ps")
                for k in range(KF):
                    MM(ps, faT[:, k, m * P:(m + 1) * P],
                       fbT[:, k, :], start=(k == 0), stop=(k == KF - 1))
                nc.vector.tensor_reduce(out=nrmax[:, m:m + 1], in_=ps,
                                        axis=mybir.AxisListType.X, op=MAX,
                                        negate=True)
                nc.scalar.activation(out=Sf8[:, m, :], in_=ps, func=EXP,
                                     bias=nrmax[:, m:m + 1], scale=1.0)
            for m in range(ML):
                nc.vector.tensor_reduce(out=rsum[:, m:m + 1], in_=Sf8[:, m, :],
                                        axis=mybir.AxisListType.X, op=ADD)

            # overlap: previous item's beta attention fills the PE while this
            # item's ACT chain runs
            if prev is not None:
                emit_attention_half(prev, 0)

            # E.T via a second matmul pass (lhsT/rhs swapped)
            for m in range(ML):
                pst = psum_pool.tile([P, L], F32, tag="ps")
                for k in range(KF):
                    MM(pst, fbT[:, k, m * P:(m + 1) * P],
                       faT[:, k, :], start=(k == 0), stop=(k == KF - 1))
                nc.vector.tensor_reduce(out=ncmax[:, m:m + 1], in_=pst,
                                        axis=mybir.AxisListType.X, op=MAX,
                                        negate=True)
                nc.scalar.activation(out=Stf8[:, m, :], in_=pst, func=EXP,
                                     bias=ncmax[:, m:m + 1], scale=1.0)
            for m in range(ML):
                nc.vector.tensor_reduce(out=csum[:, m:m + 1], in_=Stf8[:, m, :],
                                        axis=mybir.AxisListType.X, op=ADD)

            # TSb = transpose(Sf8) as fp8 DR weights [c-part]; beta lhsT
            for mt in range(ML):
                pst2 = psum_pool.tile([P, L, 2], FP8, tag="ps2", bufs=1)
                for j in range(ML):
                    nc.tensor.transpose(pst2[:, j * P:(j + 1) * P, 0],
                                        Sf8[:, j, mt * P:(mt + 1) * P], ident8)
                for s in range(2):
                    nc.scalar.copy(out=TSb[:, mt, :, s, :], in_=pst2[:, :, 0])

            # previous item's alpha attention covers the Stb/cast chains
            if prev is not None:
                emit_attention_half(prev, 1)

            # TSa = transpose(Stf8) as fp8 DR weights [a-part]; alpha lhsT
            for mt in range(ML):
                pst2 = psum_pool.tile([P, L, 2], FP8, tag="ps2", bufs=1)
                for j in range(ML):
                    nc.tensor.transpose(pst2[:, j * P:(j + 1) * P, 0],
                                        Stf8[:, j, mt * P:(mt + 1) * P], ident8)
                for s in range(2):
                    nc.scalar.copy(out=TSa[:, mt, :, s, :], in_=pst2[:, :, 0])

            prev = {"i": i, "TSb": TSb, "TSa": TSa, "a8s": a8s, "b8s": b8s,
                    "rsum": rsum, "csum": csum}
        emit_attention_half(prev, 0)
        emit_attention_half(prev, 1)
    nc.compile()
    return nc


_NC_CACHE = {}


def _get_nc(repeat=1):
    if repeat not in _NC_CACHE:
        _NC_CACHE[repeat] = _build_nc(repeat)
    return _NC_CACHE[repeat]


def _split_fp8(x):
    hi = x.astype(NP_FP8)
    lo = (x - hi.astype(np.float32)).astype(NP_FP8)
    # [BPC, L, NH, 2, 512]: per (row, nh) the hi/lo halves sit adjacent so the
    # DoubleRow ifmap slice [:, k, nh, :, :] is one contiguous 1024-run
    hi = hi.reshape(hi.shape[0], hi.shape[1], NH, 512)
    lo = lo.reshape(lo.shape[0], lo.shape[1], NH, 512)
    return np.stack([hi, lo], axis=3)


def build_in_maps(a, b, W1, b1, W2, b2):
    a = np.ascontiguousarray(np.asarray(a, dtype=np.float32))
    b = np.ascontiguousarray(np.asarray(b, dtype=np.float32))
    w1T_h = np.ascontiguousarray(np.asarray(W1, np.float32).T)
    w2T_h = np.ascontiguousarray(np.asarray(W2, np.float32).T)
    b1_h = np.ascontiguousarray(np.asarray(b1, np.float32))
    b2_h = np.ascontiguousarray(np.asarray(b2, np.float32))

    in_maps = []
    for c in range(NCORES):
        sl = slice(c * BPC, (c + 1) * BPC)
        ac, bc = a[sl], b[sl]
        in_maps.append({
            "aT": np.ascontiguousarray(ac.transpose(0, 2, 1)),
            "bT": np.ascontiguousarray(bc.transpose(0, 2, 1)),
            "a8": np.ascontiguousarray(_split_fp8(ac)),
            "b8": np.ascontiguousarray(_split_fp8(bc)),
            "w1T": w1T_h,
            "w2T": w2T_h,
            "bias1": b1_h,
            "bias2": b2_h,
        })
    return in_maps


def kernel(a, b, W1, b1, W2, b2):
    in_maps = build_in_maps(a, b, W1, b1, W2, b2)
    res = run_bass_kernel_spmd(_get_nc(), in_maps, core_ids=list(range(NCORES)))
    beta = np.concatenate([res.results[c]["beta"] for c in range(NCORES)], axis=0)
    alpha = np.concatenate([res.results[c]["alpha"] for c in range(NCORES)], axis=0)
    return beta.astype(np.float32), alpha.astype(np.float32)
